# revision 2
# baseline (speedup 1.0000x reference)
"""Causal single-head attention (B=8, S=2048, D=1024) on 8 TRN2 NeuronCores.

Sharding: data-parallel over batch -- one batch element per core, weights
replicated. Each core runs an identical Bass/Tile program:

  1. cast WQ/WK/WV and X_{q,k,v} to bf16; transpose X via PE (128x128 blocks)
  2. projections on TensorE: Q^T, K^T in [d_out, s] layout; V in [s, d_out]
  3. per 128-row query band: scores^T [k, q] = K^T-blocks^T @ Q^T (causal
     blocks only), diag block masked additively, exp on ScalarE (scale=1/32),
     P^T bf16 -> PV matmuls with P^T as the stationary operand; row sums via
     an extra N=1 ones-matmul on the same stationary tile; final 1/sum scale
     folded into the PSUM->SBUF output copy.
"""

import sys

sys.path.insert(0, "/opt/trn_rl_repo")

import numpy as np

S = 2048
D = 1024
N_CORES = 8
P = 128
SB = S // P   # 16 s-blocks
DB = D // P   # 8 d-blocks
SC = S // 512  # 4 s-chunks of 512
DC = D // 512  # 2 d-chunks of 512

_CACHE = {}


def _build():
    import concourse.bacc as bacc
    import concourse.mybir as mybir
    import concourse.tile as tile
    from concourse.masks import make_identity

    f32 = mybir.dt.float32
    bf16 = mybir.dt.bfloat16

    nc = bacc.Bacc("TRN2", target_bir_lowering=False, debug=False)

    xq = nc.dram_tensor("xq", [S, D], f32, kind="ExternalInput").ap()
    xk = nc.dram_tensor("xk", [S, D], f32, kind="ExternalInput").ap()
    xv = nc.dram_tensor("xv", [S, D], f32, kind="ExternalInput").ap()
    wq = nc.dram_tensor("wq", [D, D], f32, kind="ExternalInput").ap()
    wk = nc.dram_tensor("wk", [D, D], f32, kind="ExternalInput").ap()
    wv = nc.dram_tensor("wv", [D, D], f32, kind="ExternalInput").ap()
    out = nc.dram_tensor("out", [S, D], f32, kind="ExternalOutput").ap()

    with tile.TileContext(nc) as tc:
        with (
            tc.tile_pool(name="consts", bufs=1) as cpool,
            tc.tile_pool(name="qt", bufs=1) as qt_pool,
            tc.tile_pool(name="kt", bufs=1) as kt_pool,
            tc.tile_pool(name="vn", bufs=1) as v_pool,
        ):
            identity = cpool.tile([P, P], bf16, tag="identity")
            make_identity(nc, identity)
            ones = cpool.tile([P, 1], bf16, tag="ones")
            nc.gpsimd.memset(ones, 1.0)
            # additive causal mask for scores^T [k, q]: keep k <= q
            dmask = cpool.tile([P, P], f32, tag="dmask")
            nc.gpsimd.memset(dmask, 0.0)
            nc.gpsimd.affine_select(
                out=dmask,
                in_=dmask,
                compare_op=mybir.AluOpType.is_ge,
                fill=-1e9,
                base=0,
                # keep where (-1)*k + q >= 0
                pattern=[[1, P]],
                channel_multiplier=-1,
            )

            qt = [qt_pool.tile([P, S], bf16, tag=f"qt{d}", name=f"qt{d}") for d in range(DB)]
            kt = [kt_pool.tile([P, S], bf16, tag=f"kt{d}", name=f"kt{d}") for d in range(DB)]
            vn = [v_pool.tile([P, D], bf16, tag=f"v{s}", name=f"v{s}") for s in range(SB)]

            # ---------------- phase 1: casts, transposes, projections ------
            with (
                tc.tile_pool(name="stage", bufs=1) as stage,
                tc.tile_pool(name="wpool", bufs=1) as wpool,
                tc.tile_pool(name="xtp", bufs=1) as xt_pool,
                tc.tile_pool(name="ps1", bufs=1, space="PSUM") as ps1,
            ):
                xt = [xt_pool.tile([P, S], bf16, tag=f"xt{d}", name=f"xt{d}") for d in range(DB)]

                for x_dram, w_dram, kind in (
                    (xq, wq, "q"),
                    (xk, wk, "k"),
                    (xv, wv, "v"),
                ):
                    # load + cast weights (bf16), double-buffered across inputs
                    wtiles = []
                    for d in range(DB):
                        wf = stage.tile([P, D], f32, tag="wf", bufs=2)
                        nc.sync.dma_start(wf, w_dram[d * P : (d + 1) * P, :])
                        wb = wpool.tile([P, D], bf16, tag=f"w{d}", bufs=2)
                        nc.vector.tensor_copy(wb, wf)
                        wtiles.append(wb)

                    # load + cast + PE-transpose input into xt (bf16, [d, s])
                    for s in range(SB):
                        xf = stage.tile([P, D], f32, tag="xf", bufs=3)
                        nc.sync.dma_start(xf, x_dram[s * P : (s + 1) * P, :])
                        xb = stage.tile([P, D], bf16, tag="xb", bufs=3)
                        nc.vector.tensor_copy(xb, xf)
                        for d in range(DB):
                            tp = ps1.tile([P, P], bf16, tag="tp", bufs=4)
                            nc.tensor.transpose(
                                tp, xb[:, d * P : (d + 1) * P], identity
                            )
                            nc.vector.tensor_copy(
                                xt[d][:, s * P : (s + 1) * P], tp
                            )

                    if kind in ("q", "k"):
                        dst = qt if kind == "q" else kt
                        # dst[od][:, s] = sum_d W[d, od]^T X^T[d, s]
                        for od in range(DB):
                            for scn in range(SC):
                                pp = ps1.tile([P, 512], f32, tag="proj", bufs=2)
                                for di in range(DB):
                                    nc.tensor.matmul(
                                        pp,
                                        lhsT=wtiles[di][:, od * P : (od + 1) * P],
                                        rhs=xt[di][:, scn * 512 : (scn + 1) * 512],
                                        start=(di == 0),
                                        stop=(di == DB - 1),
                                    )
                                nc.vector.tensor_copy(
                                    dst[od][:, scn * 512 : (scn + 1) * 512], pp
                                )
                    else:
                        # V natural: vn[s][:, dc] = sum_d X^T[d, s]^T W[d, dc]
                        for s in range(SB):
                            for dc in range(DC):
                                pp = ps1.tile([P, 512], f32, tag="proj", bufs=2)
                                for di in range(DB):
                                    nc.tensor.matmul(
                                        pp,
                                        lhsT=xt[di][:, s * P : (s + 1) * P],
                                        rhs=wtiles[di][:, dc * 512 : (dc + 1) * 512],
                                        start=(di == 0),
                                        stop=(di == DB - 1),
                                    )
                                nc.vector.tensor_copy(
                                    vn[s][:, dc * 512 : (dc + 1) * 512], pp
                                )

            # ---------------- phase 2: causal attention per q band ---------
            with (
                tc.tile_pool(name="ptp", bufs=1) as pt_pool,
                tc.tile_pool(name="outp", bufs=1) as out_pool,
                tc.tile_pool(name="ps_sc", bufs=1, space="PSUM") as ps_sc,
                tc.tile_pool(name="ps_pv", bufs=1, space="PSUM") as ps_pv,
            ):
                for qi in range(SB):
                    nkb = qi + 1
                    pts = []
                    for kb in range(nkb):
                        sc = ps_sc.tile([P, P], f32, tag="sc", bufs=4)
                        for di in range(DB):
                            nc.tensor.matmul(
                                sc,
                                lhsT=kt[di][:, kb * P : (kb + 1) * P],
                                rhs=qt[di][:, qi * P : (qi + 1) * P],
                                start=(di == 0),
                                stop=(di == DB - 1),
                            )
                        if kb == qi:
                            nc.vector.tensor_add(sc, sc, dmask)
                        pt = pt_pool.tile([P, P], bf16, tag="pt", bufs=20)
                        nc.scalar.activation(
                            pt, sc, mybir.ActivationFunctionType.Exp,
                            scale=1.0 / 32.0,
                        )
                        pts.append(pt)

                    pv1 = ps_pv.tile([P, 512], f32, tag="pv1", bufs=1)
                    pv2 = ps_pv.tile([P, 512], f32, tag="pv2", bufs=1)
                    rowsum = ps_pv.tile([P, 1], f32, tag="rowsum", bufs=2)
                    for kb in range(nkb):
                        st = kb == 0
                        sp = kb == nkb - 1
                        nc.tensor.matmul(
                            pv1, lhsT=pts[kb], rhs=vn[kb][:, 0:512],
                            start=st, stop=sp,
                        )
                        nc.tensor.matmul(
                            pv2, lhsT=pts[kb], rhs=vn[kb][:, 512:1024],
                            start=st, stop=sp,
                        )
                        nc.tensor.matmul(
                            rowsum, lhsT=pts[kb], rhs=ones, start=st, stop=sp,
                        )

                    recip = out_pool.tile([P, 1], f32, tag="recip", bufs=2)
                    nc.vector.reciprocal(recip, rowsum)
                    ob = out_pool.tile([P, D], f32, tag="ob", bufs=2)
                    nc.vector.tensor_scalar_mul(ob[:, 0:512], pv1, recip)
                    nc.vector.tensor_scalar_mul(ob[:, 512:1024], pv2, recip)
                    nc.sync.dma_start(out[qi * P : (qi + 1) * P, :], ob)

    nc.compile()
    return nc


def _get_nc():
    if "nc" not in _CACHE:
        _CACHE["nc"] = _build()
    return _CACHE["nc"]


def _run(in_maps, trace=False):
    from concourse.bass_utils import run_bass_kernel_spmd

    nc = _get_nc()
    return run_bass_kernel_spmd(
        nc, in_maps, core_ids=list(range(N_CORES)), trace=trace
    )


def _in_maps(inputs):
    fq = np.ascontiguousarray(np.asarray(inputs["inputs_for_queries"], np.float32))
    fk = np.ascontiguousarray(np.asarray(inputs["inputs_for_keys"], np.float32))
    fv = np.ascontiguousarray(np.asarray(inputs["inputs_for_values"], np.float32))
    WQ = np.ascontiguousarray(np.asarray(inputs["WQ"], np.float32))
    WK = np.ascontiguousarray(np.asarray(inputs["WK"], np.float32))
    WV = np.ascontiguousarray(np.asarray(inputs["WV"], np.float32))
    return [
        {
            "xq": fq[c],
            "xk": fk[c],
            "xv": fv[c],
            "wq": WQ,
            "wk": WK,
            "wv": WV,
        }
        for c in range(N_CORES)
    ]


def kernel(**inputs) -> np.ndarray:
    res = _run(_in_maps(inputs))
    return np.stack([res.results[c]["out"] for c in range(N_CORES)], axis=0)
